# revision 20
# baseline (speedup 1.0000x reference)
"""Trainium2 Bass kernel for the MoE-routing random-feature ridge problem.

Strategy (8 NeuronCores, feature-parallel SPMD):
  - Atoms are grouped by element (stable sort preserves the sorted molID
    order) and padded per element to a multiple of CH=512.  Every core
    processes all atoms but owns a 512-wide slice of the 4096 random
    features (weights W/bias are column-sliced per core on the host).
  - Per 512-atom chunk (single element e), big matmuls run in fp8e4m3
    with DoubleRow perf mode (two 128-deep k-tiles per instruction):
      PT16 = (16*reductors[e])^T @ gto_chunk^T        [256, 512]  (PE, fp8 DR)
      PW   = PT16^T @ (16*W[e][:, fslice]) = 256*phase            (PE, fp8 DR)
      Fw   = fused-DVE: (PW/256 + c_bc) range-wrapped into [-pi, pi]
      F    = sin(Fw)  (ScalarE, bf16 out)
      Z[mt]+= ST_chunk^T @ F                          (PE bf16, accumulated
             across chunks directly in a PSUM bank per mol tile; W=3 banks
             live at a time, spilled once per tile to fp8 zr by GpSimd)
    The x16 input scaling keeps reductors/W out of the fp8 subnormal
    range; the /256 is folded into the custom wrap DVE op (s0).
  - Each finished 128-mol tile of Z (fp8) is AllGathered immediately
    (8 staged 1-tile collectives overlap phase-1 compute; chunk windows
    are uncapped so only tile 7 completes at phase-1 end).
  - Phase 2: gathered panels land in one [128, core, mtile, 512] fp8
    SBUF tile via one big DMA per collective (prefetched 3 groups
    behind the AG stream).  The core's 512-row slice of Z^T Z runs as
    fp8 DoubleRow chains: A = k-pairs (0,1)(2,3)(4,5) into PSUM ->
    f32 SBUF, Z^T Y (from fp8 zr), then the late pair (6,7) is added
    with DVE and stored as bf16 (one DMA per 512-col block).
  - Host applies scale^2 = 2/NFEAT, adds lambda*I, upcasts, and
    concatenates the per-core slices.

Measured: see test.py output history; gate is rel err < 2e-2.
"""

import sys

if "/opt/trn_rl_repo" not in sys.path:
    sys.path.insert(0, "/opt/trn_rl_repo")

import numpy as np

import concourse.bacc as bacc
import concourse.mybir as mybir
import concourse.tile as tile
from concourse import bass_utils

NCORES = 8
NATOMS = 16384
NMOL = 1024
REP = 512
PROJ = 256
NFEAT = 4096
NELEM = 4
LLAMBDA = 1e-6

CH = 512           # atoms per chunk
NF_LOC = NFEAT // NCORES   # features per core (512)
MOLT = NMOL // 128          # mol tiles (8)

F32 = mybir.dt.float32
F32R = mybir.dt.float32r
BF16 = mybir.dt.bfloat16
FP8 = mybir.dt.float8e4
DR = mybir.MatmulPerfMode.DoubleRow

# --- fused (in0 + in1) + range-wrap custom DVE op ---------------------------
from concourse import dve_ops as _dve_ops
from concourse.dve_spec import Spec as _Spec, Src0 as _Src0, Src1 as _Src1
from concourse.dve_spec import C1 as _C1, C2 as _C2, _has_src1, lower as _dve_lower
from concourse.dve_uop import DveOpSpec as _DveOpSpec

_A2RW_NAME = "ADD2_RANGE_WRAP_ANT"
if _A2RW_NAME not in _dve_ops._SUB_OPCODE_FOR_NAME:
    _y = _Src0 + _Src1
    _a2_spec = _Spec(
        body=_y + _C2 * ((_y < -_C1) - (_y > _C1)),
        reference=lambda in0, in1, s0, s1, imm2: (in0 + in1)
        + imm2
        * (
            ((in0 + in1) < -s1).astype(np.float32)
            - ((in0 + in1) > s1).astype(np.float32)
        ),
    )
    _shas = {}
    for _ver in ("v3", "v4"):
        _tmp = _DveOpSpec(name=_A2RW_NAME, opcode=1,
                          uops=_dve_lower(_a2_spec, ver=_ver),
                          rd1_en=_has_src1(_a2_spec))
        _shas[_ver] = _tmp.sha(_ver)
    ADD2_RANGE_WRAP = _dve_ops.DveOp(_A2RW_NAME, _a2_spec, subdim=False, uops_sha=_shas)
    _dve_ops.OPS.append(ADD2_RANGE_WRAP)
    _dve_ops.CUSTOM_DVE_SPECS[_A2RW_NAME] = _a2_spec
    _dve_ops._SUB_OPCODE_FOR_NAME[_A2RW_NAME] = (
        max(_dve_ops._SUB_OPCODE_FOR_NAME.values()) + 1
    )
else:
    ADD2_RANGE_WRAP = next(o for o in _dve_ops.OPS if o.name == _A2RW_NAME)

_SAW_NAME = "SCALE_ADD_RANGE_WRAP_ANT"
if _SAW_NAME not in _dve_ops._SUB_OPCODE_FOR_NAME:
    from concourse.dve_spec import C0 as _C0
    _ys = _Src0 * _C0 + _Src1
    _saw_spec = _Spec(
        body=_ys + _C2 * ((_ys < -_C1) - (_ys > _C1)),
        reference=lambda in0, in1, s0, s1, imm2: (in0 * s0 + in1)
        + imm2
        * (
            ((in0 * s0 + in1) < -s1).astype(np.float32)
            - ((in0 * s0 + in1) > s1).astype(np.float32)
        ),
    )
    _shas2 = {}
    for _ver in ("v3", "v4"):
        _tmp2 = _DveOpSpec(name=_SAW_NAME, opcode=1,
                           uops=_dve_lower(_saw_spec, ver=_ver),
                           rd1_en=_has_src1(_saw_spec))
        _shas2[_ver] = _tmp2.sha(_ver)
    SCALE_ADD_RANGE_WRAP = _dve_ops.DveOp(_SAW_NAME, _saw_spec, subdim=False, uops_sha=_shas2)
    _dve_ops.OPS.append(SCALE_ADD_RANGE_WRAP)
    _dve_ops.CUSTOM_DVE_SPECS[_SAW_NAME] = _saw_spec
    _dve_ops._SUB_OPCODE_FOR_NAME[_SAW_NAME] = (
        max(_dve_ops._SUB_OPCODE_FOR_NAME.values()) + 1
    )
else:
    SCALE_ADD_RANGE_WRAP = next(o for o in _dve_ops.OPS if o.name == _SAW_NAME)

_cache = {}


def _plan(charges, molIDs):
    """Host-side chunking plan from charges/molIDs (static per compile)."""
    charges = np.asarray(charges)
    molIDs = np.asarray(molIDs)
    assert np.all(np.diff(molIDs) >= 0), "molIDs must be sorted"
    perm = np.argsort(charges, kind="stable")
    mol_p = molIDs[perm]
    chg_p = charges[perm]

    # padded element groups
    counts = np.bincount(charges, minlength=NELEM)
    padded = [int(np.ceil(c / CH) * CH) for c in counts]
    A_pad = int(sum(padded))
    n_chunks = A_pad // CH

    # index into permuted arrays for each padded slot (-1 = padding)
    slot_idx = np.full(A_pad, -1, dtype=np.int64)
    src_off = 0
    dst_off = 0
    for e in range(NELEM):
        c = int(counts[e])
        slot_idx[dst_off:dst_off + c] = np.arange(src_off, src_off + c)
        src_off += c
        dst_off += padded[e]

    chunk_elem = []
    chunk_m0 = []
    W_need = 1
    for c in range(n_chunks):
        sl = slot_idx[c * CH:(c + 1) * CH]
        real = sl >= 0
        if real.any():
            mols = mol_p[sl[real]]
            t_lo = int(mols.min()) // 128
            t_hi = int(mols.max()) // 128
            W_need = max(W_need, t_hi - t_lo + 1)
            chunk_m0.append(t_lo)
            e = int(chg_p[sl[real][0]])
        else:
            chunk_m0.append(0)
            e = int(np.searchsorted(np.cumsum(padded), c * CH, side="right"))
        chunk_elem.append(e)
    W = W_need

    # nonzero (k-tile, wt) blocks of ST per chunk + ST data
    st_blocks = []   # list per chunk: list of (kt, wt) nonzero
    ST = np.zeros((n_chunks, CH, W * 128), dtype=np.float32)
    for c in range(n_chunks):
        sl = slot_idx[c * CH:(c + 1) * CH]
        real = np.nonzero(sl >= 0)[0]
        blocks = set()
        if len(real):
            ml = mol_p[sl[real]] - chunk_m0[c] * 128
            ok = (ml >= 0) & (ml < W * 128)
            ST[c, real[ok], ml[ok]] = 1.0
            for a, m in zip(real[ok], ml[ok]):
                blocks.add((int(a) // 128, int(m) // 128))
        st_blocks.append(sorted(blocks))

    chunk_real = [int((slot_idx[c * CH:(c + 1) * CH] >= 0).sum())
                  for c in range(n_chunks)]
    return dict(perm=perm, slot_idx=slot_idx, A_pad=A_pad, n_chunks=n_chunks,
                chunk_elem=chunk_elem, chunk_m0=chunk_m0, W=W, ST=ST,
                st_blocks=st_blocks, chunk_real=chunk_real)


def _build(plan):
    n_chunks = plan["n_chunks"]
    W = plan["W"]
    chunk_elem = plan["chunk_elem"]
    chunk_m0 = plan["chunk_m0"]
    st_blocks = plan["st_blocks"]

    nc = bacc.Bacc(num_devices=NCORES)
    gto_d = nc.dram_tensor("gto_swz", [n_chunks, 128, 4, CH], FP8, kind="ExternalInput")
    st_d = nc.dram_tensor("st_swz", [n_chunks, 128, 4 * W * 128], BF16, kind="ExternalInput")
    red_d = nc.dram_tensor("red_swz", [128, NELEM * 4, PROJ], FP8, kind="ExternalInput")
    w_d = nc.dram_tensor("w_swz", [128, NELEM * 2, NF_LOC], FP8, kind="ExternalInput")
    c_d = nc.dram_tensor("c_swz", [1, NELEM * NF_LOC], F32, kind="ExternalInput")
    y_d = nc.dram_tensor("y_swz", [128, MOLT], F32, kind="ExternalInput")
    ztz_d = nc.dram_tensor("ztz", [NF_LOC, NFEAT], BF16, kind="ExternalOutput")
    zty_d = nc.dram_tensor("zty", [NF_LOC, 1], F32, kind="ExternalOutput")

    with tile.TileContext(nc) as tc:
        with (
            tc.tile_pool(name="const", bufs=1) as constp,
            tc.tile_pool(name="zacc", bufs=1) as zaccp,
            tc.tile_pool(name="dram", bufs=1, space="DRAM") as dramp,
        ):
            red_sb = constp.tile([128, NELEM * 4, PROJ], FP8, tag="red")
            w_sb = constp.tile([128, NELEM * 2, NF_LOC], FP8, tag="w")
            c_sb = constp.tile([1, NELEM * NF_LOC], F32, tag="c")
            c_bc = constp.tile([128, NELEM * NF_LOC], F32, tag="cbc")
            y_sb = constp.tile([128, MOLT], F32, tag="y")
            # load order: red first (chunk-0 PT needs it), then the W slice for
            # the first element processed, then c/y, then the remaining W.
            order = sorted(range(n_chunks), key=lambda c: (chunk_m0[c], c))
            e_first = chunk_elem[order[0]]
            for q in [e_first] + [q for q in range(4) if q != e_first]:
                nc.sync.dma_start(out=red_sb[:, q * 4:(q + 1) * 4, :],
                                  in_=red_d[:, q * 4:(q + 1) * 4, :])
            nc.sync.dma_start(out=w_sb[:, e_first * 2:e_first * 2 + 2, :],
                              in_=w_d[:, e_first * 2:e_first * 2 + 2, :])
            nc.sync.dma_start(out=c_sb[:], in_=c_d[:])
            nc.gpsimd.partition_broadcast(c_bc[:], c_sb[:])
            nc.sync.dma_start(out=y_sb[:], in_=y_d[:])
            for e in range(NELEM):
                if e == e_first:
                    continue
                nc.sync.dma_start(out=w_sb[:, e * 2:e * 2 + 2, :],
                                  in_=w_d[:, e * 2:e * 2 + 2, :])

            z_sb = zaccp.tile([128, NMOL // 128 * NF_LOC], F32, tag="z")     # [128, 4096]
            zr_sb = zaccp.tile([128, MOLT, NF_LOC], FP8, tag="zr")
            nc.vector.memset(z_sb[:], 0.0)

            in_b = dramp.tile([NMOL, NF_LOC], FP8, tag="agin")
            GB = list(range(MOLT + 1))  # one mol tile per AG group
            NG = len(GB) - 1
            ag_bs = [
                dramp.tile([NCORES * (GB[g + 1] - GB[g]) * 128, NF_LOC], FP8,
                           addr_space="Shared", tag=f"agout{g}", name=f"ag_b_{g}")
                for g in range(NG)
            ]

            # ---------------- phase 1: chunks (m0-sorted) ----------------
            # last order-position touching each mol group
            group_last = [0] * NG
            for pos, ci in enumerate(order):
                if not st_blocks[ci]:
                    continue
                wts = {chunk_m0[ci] + wt for (kt, wt) in st_blocks[ci]}
                for mt in wts:
                    g = next(i for i in range(NG) if GB[i] <= mt < GB[i + 1])
                    group_last[g] = max(group_last[g], pos)
            for g in range(NG):  # groups complete monotonically
                group_last[g] = max(group_last[:g + 1])
            group_at = {}
            for g in range(NG):
                group_at.setdefault(group_last[g], []).append(g)

            _panel_cm = tc.tile_pool(name="panel", bufs=1)
            panelp = _panel_cm.__enter__()
            pan_all = panelp.tile([128, NCORES, MOLT, NF_LOC], FP8, tag="pan")

            def _mk_loader(g):
                def _ld():
                    nc.sync.dma_start(
                        out=pan_all[:, :, g, :],
                        in_=ag_bs[g][:].rearrange("(t p) c -> p t c", p=128),
                    )
                return _ld

            pang_loaders = {g: _mk_loader(g) for g in range(NG)}

            def emit_group_tail(g):
                for k in range(GB[g], GB[g + 1]):
                    nc.gpsimd.tensor_copy(
                        zr_sb[:, k, :],
                        z_sb[:, k * NF_LOC:(k + 1) * NF_LOC],
                    )
                    nc.sync.dma_start(
                        out=in_b[k * 128:(k + 1) * 128, :],
                        in_=zr_sb[:, k, :],
                    )
                nc.gpsimd.collective_compute(
                    "AllGather",
                    mybir.AluOpType.bypass,
                    replica_groups=[list(range(NCORES))],
                    ins=[in_b[GB[g] * 128:GB[g + 1] * 128, :].opt()],
                    outs=[ag_bs[g][:].opt()],
                )
                # prefetch the panel block of an AG that finished a while ago
                # (g-3 keeps the sync queue from blocking on the AG sem)
                if g - 3 in pang_loaders:
                    pang_loaders.pop(g - 3)()
                if g == NG - 1 and 5 in pang_loaders:
                    pang_loaders.pop(5)()

            with (
                tc.tile_pool(name="gtop", bufs=3) as gtop,
                tc.tile_pool(name="stp", bufs=3) as stp,
                tc.tile_pool(name="ptp", bufs=3) as ptp,
                tc.tile_pool(name="fp", bufs=3) as fpool,
                tc.tile_pool(name="ppt", bufs=2, space="PSUM") as ppt,
                tc.tile_pool(name="pf", bufs=3, space="PSUM") as pf,
                tc.tile_pool(name="pz", bufs=3, space="PSUM") as pz,
            ):
                n_real = plan["chunk_real"]
                contribs = {}
                for pos, ci in enumerate(order):
                    for (kt, wt) in sorted(set(st_blocks[ci])):
                        mt = chunk_m0[ci] + wt
                        lst = contribs.setdefault(mt, [])
                        if not lst or lst[-1] != pos:
                            lst.append(pos)
                zb_start = set()
                zb_stop = set()
                for mt, lst in contribs.items():
                    for j, pos in enumerate(lst):
                        if j % 2 == 0:
                            zb_start.add((mt, pos))
                        if j % 2 == 1 or j == len(lst) - 1:
                            zb_stop.add((mt, pos))
                z_ps_live = {}
                for pos, ci in enumerate(order):
                    e = chunk_elem[ci]
                    m0 = chunk_m0[ci]
                    if n_real[ci] == 0:
                        for g in group_at.get(pos, []):
                            emit_group_tail(g)
                        continue
                    mts = (n_real[ci] + 127) // 128   # live atom tiles
                    gto_t = gtop.tile([128, 4, CH], FP8, tag="gto")
                    nc.sync.dma_start(out=gto_t[:], in_=gto_d[ci, :, :, :])
                    st_t = stp.tile([128, 4 * W * 128], BF16, tag="st")
                    if st_blocks[ci]:
                        nc.sync.dma_start(out=st_t[:], in_=st_d[ci, :, :])

                    # PT [256, 512] (x16) -> pt_sb [128, 2, 512] fp8
                    pt_sb = ptp.tile([128, 2, CH], FP8, tag="pt")
                    for mp in range(2):  # proj tile
                        pt_ps = ppt.tile([128, CH], F32, tag="ptps")
                        for ktp in range(0, 4, 2):  # rep k tile pairs
                            nc.tensor.matmul(
                                pt_ps[:],
                                red_sb[:, e * 4 + ktp:e * 4 + ktp + 2,
                                       mp * 128:mp * 128 + 128],
                                gto_t[:, ktp:ktp + 2, :],
                                start=(ktp == 0), stop=(ktp == 2),
                                perf_mode=DR,
                            )
                        if pos % 2 == 0:
                            nc.scalar.copy(pt_sb[:, mp, :], pt_ps[:])
                        else:
                            nc.vector.tensor_copy(pt_sb[:, mp, :], pt_ps[:])

                    # feats F [512 atoms, 512 feats] -> f_sb [128, 4*512]
                    f_sb = fpool.tile([128, 4 * NF_LOC], BF16, tag="f")
                    for mt in range(mts):  # atom tile
                        f_ps = pf.tile([128, NF_LOC], F32, tag="fps")
                        nc.tensor.matmul(
                            f_ps[:],
                            pt_sb[:, 0:2, mt * 128:mt * 128 + 128],
                            w_sb[:, e * 2:e * 2 + 2, :],
                            start=True, stop=True,
                            perf_mode=DR,
                        )
                        fw = fpool.tile([128, NF_LOC], F32, tag="fw")
                        nc.vector._custom_dve(
                            SCALE_ADD_RANGE_WRAP, out=fw[:], in0=f_ps[:],
                            in1=c_bc[:, e * NF_LOC:(e + 1) * NF_LOC],
                            s0=float(1.0 / 256.0),
                            s1=float(np.pi), imm2=float(2 * np.pi),
                        )
                        nc.scalar.activation(
                            f_sb[:, mt * NF_LOC:(mt + 1) * NF_LOC], fw[:],
                            mybir.ActivationFunctionType.Sin,
                        )

                    # Z += ST^T @ F; PSUM tile accumulates 2 chunks per flush
                    for wt in range(W):
                        kts = [kt for (kt, w2) in st_blocks[ci] if w2 == wt]
                        if not kts:
                            continue
                        mt_out = m0 + wt
                        first = (mt_out, pos) in zb_start
                        last = (mt_out, pos) in zb_stop
                        if first:
                            z_ps_live[mt_out] = pz.tile([128, NF_LOC], F32, tag="zps", name=f"zps_{mt_out}_{pos}")
                        z_ps = z_ps_live[mt_out]
                        for i, kt in enumerate(kts):
                            nc.tensor.matmul(
                                z_ps[:],
                                st_t[:, (kt * W + wt) * 128:(kt * W + wt) * 128 + 128],
                                f_sb[:, kt * NF_LOC:(kt + 1) * NF_LOC],
                                start=(first and i == 0),
                                stop=(last and i == len(kts) - 1),
                            )
                        if last:
                            del z_ps_live[mt_out]
                            nc.vector.tensor_add(
                                z_sb[:, mt_out * NF_LOC:(mt_out + 1) * NF_LOC],
                                z_sb[:, mt_out * NF_LOC:(mt_out + 1) * NF_LOC],
                                z_ps[:],
                            )

                    for g in group_at.get(pos, []):
                        emit_group_tail(g)

            # ---------------- phase 2: ZTZ slice (A: k<6, B: k=6,7) ----------------
            with (
                tc.tile_pool(name="osb", bufs=NCORES) as osbp,
                tc.tile_pool(name="obsb", bufs=NCORES) as obsbp,
                tc.tile_pool(name="pztz", bufs=4, space="PSUM") as pztz,
                tc.tile_pool(name="pztzB", bufs=3, space="PSUM") as pztzB,
                tc.tile_pool(name="pzty", bufs=1, space="PSUM") as pzty,
            ):
                for g in sorted(pang_loaders):   # not prefetched during phase 1
                    pang_loaders.pop(g)()

                KA = 6  # k-tiles available once AG_0..AG_2 land (phase-1 end)
                o_sbs = {}
                for n in range(NCORES):
                    o_sb = osbp.tile([128, 4, NF_LOC], F32, tag="osb",
                                     name=f"o_sb_{n}")
                    o_sbs[n] = o_sb
                    for m in range(4):
                        ztz_ps = pztz.tile([128, NF_LOC], F32, tag="ztzpsA",
                                           name=f"ztzA_{n}_{m}")
                        for kp in range(0, KA, 2):
                            nc.tensor.matmul(
                                ztz_ps[:],
                                zr_sb[:, kp:kp + 2, m * 128:m * 128 + 128],
                                pan_all[:, n, kp:kp + 2, :],
                                start=(kp == 0), stop=(kp == KA - 2),
                                perf_mode=DR,
                            )
                        nc.scalar.copy(o_sb[:, m, :], ztz_ps[:])
                # ZtY on PE while the last gather flies
                zty_ps = pzty.tile([128, 4], F32, tag="ztyps")
                for m in range(4):
                    for k in range(MOLT):
                        nc.tensor.matmul(
                            zty_ps[:, m:m + 1],
                            z_sb[:, k * NF_LOC + m * 128: k * NF_LOC + m * 128 + 128],
                            y_sb[:, k:k + 1],
                            start=(k == 0), stop=(k == MOLT - 1),
                        )
                zty_sb = zaccp.tile([128, 4], F32, tag="ztysb")
                nc.vector.tensor_copy(zty_sb[:], zty_ps[:])
                nc.sync.dma_start(
                    out=zty_d[:].rearrange("(m p) o -> p (m o)", p=128),
                    in_=zty_sb[:],
                )
                for n in range(NCORES):
                    o_sb = o_sbs[n]
                    ob_sb = obsbp.tile([128, 4, NF_LOC], BF16, tag="obsb",
                                       name=f"ob_sb_{n}")
                    for m in range(4):
                        ztz_ps = pztzB.tile([128, NF_LOC], F32, tag="ztzpsB",
                                            name=f"ztzB_{n}_{m}")
                        nc.tensor.matmul(
                            ztz_ps[:],
                            zr_sb[:, KA:KA + 2, m * 128:m * 128 + 128],
                            pan_all[:, n, KA:KA + 2, :],
                            start=True, stop=True,
                            perf_mode=DR,
                        )
                        nc.vector.tensor_add(
                            ob_sb[:, m, :], o_sb[:, m, :], ztz_ps[:],
                        )
                    nc.sync.dma_start(
                        out=ztz_d[:, n * NF_LOC:(n + 1) * NF_LOC]
                            .rearrange("(m p) c -> p m c", p=128),
                        in_=ob_sb[:],
                    )
            _panel_cm.__exit__(None, None, None)
    nc.finalize()
    return nc


def _prep_inputs(gto, reductors, W_in, b, Y, plan):
    n_chunks = plan["n_chunks"]
    A_pad = plan["A_pad"]
    slot_idx = plan["slot_idx"]
    W = plan["W"]

    gto_p = np.zeros((A_pad, REP), dtype=np.float32)
    real = slot_idx >= 0
    gto_p[real] = np.asarray(gto)[plan["perm"][slot_idx[real]]]
    # [A_pad, REP] -> [n_chunks, 128(rep-part), 4, CH]
    gto_swz = np.ascontiguousarray(
        gto_p.reshape(n_chunks, CH, 4, 128).transpose(0, 3, 2, 1)
    ).astype(mybir.dt.np(mybir.dt.float8e4))

    st_swz = np.ascontiguousarray(
        plan["ST"].reshape(n_chunks, 4, 128, W * 128).transpose(0, 2, 1, 3)
    ).reshape(n_chunks, 128, 4 * W * 128).astype(mybir.dt.np(BF16))

    red_swz = np.ascontiguousarray(
        (np.asarray(reductors) * np.float32(16.0))
        .reshape(NELEM, 4, 128, PROJ).transpose(2, 0, 1, 3)
    ).reshape(128, NELEM * 4, PROJ).astype(mybir.dt.np(mybir.dt.float8e4))

    c_full = np.mod(np.asarray(b) + np.pi / 2 + np.pi, 2 * np.pi) - np.pi  # [-pi, pi)

    W_np = np.asarray(W_in)
    in_maps = []
    for d in range(NCORES):
        fsl = slice(d * NF_LOC, (d + 1) * NF_LOC)
        w_swz = np.ascontiguousarray(
            (W_np[:, :, fsl] * np.float32(16.0))
            .reshape(NELEM, 2, 128, NF_LOC).transpose(2, 0, 1, 3)
        ).reshape(128, NELEM * 2, NF_LOC)
        c_swz = np.ascontiguousarray(c_full[:, fsl]).reshape(1, NELEM * NF_LOC)
        in_maps.append({
            "gto_swz": gto_swz,
            "st_swz": st_swz,
            "red_swz": red_swz,
            "w_swz": w_swz.astype(mybir.dt.np(mybir.dt.float8e4)),
            "c_swz": c_swz.astype(np.float32),
            "y_swz": np.ascontiguousarray(
                np.asarray(Y).reshape(MOLT, 128).T
            ).astype(np.float32),
        })
    return in_maps


def _get_built(charges, molIDs):
    key = (hash(np.asarray(charges).tobytes()), hash(np.asarray(molIDs).tobytes()))
    if key not in _cache:
        plan = _plan(charges, molIDs)
        nc = _build(plan)
        _cache[key] = (plan, nc)
    return _cache[key]


def run(gto, reductors, W, b, Y, charges, molIDs, trace=False, tmpdir=None):
    plan, nc = _get_built(charges, molIDs)
    in_maps = _prep_inputs(gto, reductors, W, b, Y, plan)
    res = bass_utils.run_bass_kernel_spmd(
        nc, in_maps, core_ids=list(range(NCORES)), trace=trace, tmpdir=tmpdir,
    )
    scale2 = 2.0 / NFEAT
    scale = np.float32(np.sqrt(scale2))
    ztz = np.concatenate([res.results[d]["ztz"].astype(np.float32) for d in range(NCORES)], axis=0)
    zty = np.concatenate([res.results[d]["zty"] for d in range(NCORES)], axis=0)
    ztz = ztz * np.float32(scale2)
    ztz[np.arange(NFEAT), np.arange(NFEAT)] += np.float32(LLAMBDA)
    zty = zty * scale
    out = np.concatenate([ztz, zty], axis=1).astype(np.float32)
    return out, res


def kernel(gto, reductors, W, b, Y, charges, molIDs):
    out, _ = run(gto, reductors, W, b, Y, charges, molIDs)
    return out



# revision 21
# speedup vs baseline: 1.0696x; 1.0696x over previous
"""Trainium2 Bass kernel for the MoE-routing random-feature ridge problem.

Strategy (8 NeuronCores, feature-parallel SPMD):
  - Atoms are grouped by element (stable sort preserves the sorted molID
    order) and padded per element to a multiple of CH=512.  Every core
    processes all atoms but owns a 512-wide slice of the 4096 random
    features (weights W/bias are column-sliced per core on the host).
  - Per 512-atom chunk (single element e), big matmuls run in fp8e4m3
    with DoubleRow perf mode (two 128-deep k-tiles per instruction):
      PT16 = (16*reductors[e])^T @ gto_chunk^T        [256, 512]  (PE, fp8 DR)
      PW   = PT16^T @ (16*W[e][:, fslice]) = 256*phase            (PE, fp8 DR)
      Fw   = fused-DVE: (PW/256 + c_bc) range-wrapped into [-pi, pi]
      F    = sin(Fw)  (ScalarE, bf16 out)
      Z[mt]+= ST_chunk^T @ F                          (PE bf16, accumulated
             across chunks directly in a PSUM bank per mol tile; W=3 banks
             live at a time, spilled once per tile to fp8 zr by GpSimd)
    The x16 input scaling keeps reductors/W out of the fp8 subnormal
    range; the /256 is folded into the custom wrap DVE op (s0).
  - Each finished 128-mol tile of Z (fp8) is AllGathered immediately
    (8 staged 1-tile collectives overlap phase-1 compute; chunk windows
    are uncapped so only tile 7 completes at phase-1 end).
  - Phase 2: gathered panels land in one [128, core, mtile, 512] fp8
    SBUF tile via one big DMA per collective (prefetched 3 groups
    behind the AG stream).  The core's 512-row slice of Z^T Z runs as
    fp8 DoubleRow chains: A = k-pairs (0,1)(2,3)(4,5) into PSUM ->
    f32 SBUF, Z^T Y (from fp8 zr), then the late pair (6,7) is added
    with DVE and stored as bf16 (one DMA per 512-col block).
  - Host applies scale^2 = 2/NFEAT, adds lambda*I, upcasts, and
    concatenates the per-core slices.

Measured: see test.py output history; gate is rel err < 2e-2.
"""

import sys

if "/opt/trn_rl_repo" not in sys.path:
    sys.path.insert(0, "/opt/trn_rl_repo")

import numpy as np

import concourse.bacc as bacc
import concourse.mybir as mybir
import concourse.tile as tile
from concourse import bass_utils

NCORES = 8
NATOMS = 16384
NMOL = 1024
REP = 512
PROJ = 256
NFEAT = 4096
NELEM = 4
LLAMBDA = 1e-6

CH = 512           # atoms per chunk
NF_LOC = NFEAT // NCORES   # features per core (512)
MOLT = NMOL // 128          # mol tiles (8)

F32 = mybir.dt.float32
F32R = mybir.dt.float32r
BF16 = mybir.dt.bfloat16
FP8 = mybir.dt.float8e4
DR = mybir.MatmulPerfMode.DoubleRow

# --- fused (in0 + in1) + range-wrap custom DVE op ---------------------------
from concourse import dve_ops as _dve_ops
from concourse.dve_spec import Spec as _Spec, Src0 as _Src0, Src1 as _Src1
from concourse.dve_spec import C1 as _C1, C2 as _C2, _has_src1, lower as _dve_lower
from concourse.dve_uop import DveOpSpec as _DveOpSpec

_A2RW_NAME = "ADD2_RANGE_WRAP_ANT"
if _A2RW_NAME not in _dve_ops._SUB_OPCODE_FOR_NAME:
    _y = _Src0 + _Src1
    _a2_spec = _Spec(
        body=_y + _C2 * ((_y < -_C1) - (_y > _C1)),
        reference=lambda in0, in1, s0, s1, imm2: (in0 + in1)
        + imm2
        * (
            ((in0 + in1) < -s1).astype(np.float32)
            - ((in0 + in1) > s1).astype(np.float32)
        ),
    )
    _shas = {}
    for _ver in ("v3", "v4"):
        _tmp = _DveOpSpec(name=_A2RW_NAME, opcode=1,
                          uops=_dve_lower(_a2_spec, ver=_ver),
                          rd1_en=_has_src1(_a2_spec))
        _shas[_ver] = _tmp.sha(_ver)
    ADD2_RANGE_WRAP = _dve_ops.DveOp(_A2RW_NAME, _a2_spec, subdim=False, uops_sha=_shas)
    _dve_ops.OPS.append(ADD2_RANGE_WRAP)
    _dve_ops.CUSTOM_DVE_SPECS[_A2RW_NAME] = _a2_spec
    _dve_ops._SUB_OPCODE_FOR_NAME[_A2RW_NAME] = (
        max(_dve_ops._SUB_OPCODE_FOR_NAME.values()) + 1
    )
else:
    ADD2_RANGE_WRAP = next(o for o in _dve_ops.OPS if o.name == _A2RW_NAME)

_SAW_NAME = "SCALE_ADD_RANGE_WRAP_ANT"
if _SAW_NAME not in _dve_ops._SUB_OPCODE_FOR_NAME:
    from concourse.dve_spec import C0 as _C0
    _ys = _Src0 * _C0 + _Src1
    _saw_spec = _Spec(
        body=_ys + _C2 * ((_ys < -_C1) - (_ys > _C1)),
        reference=lambda in0, in1, s0, s1, imm2: (in0 * s0 + in1)
        + imm2
        * (
            ((in0 * s0 + in1) < -s1).astype(np.float32)
            - ((in0 * s0 + in1) > s1).astype(np.float32)
        ),
    )
    _shas2 = {}
    for _ver in ("v3", "v4"):
        _tmp2 = _DveOpSpec(name=_SAW_NAME, opcode=1,
                           uops=_dve_lower(_saw_spec, ver=_ver),
                           rd1_en=_has_src1(_saw_spec))
        _shas2[_ver] = _tmp2.sha(_ver)
    SCALE_ADD_RANGE_WRAP = _dve_ops.DveOp(_SAW_NAME, _saw_spec, subdim=False, uops_sha=_shas2)
    _dve_ops.OPS.append(SCALE_ADD_RANGE_WRAP)
    _dve_ops.CUSTOM_DVE_SPECS[_SAW_NAME] = _saw_spec
    _dve_ops._SUB_OPCODE_FOR_NAME[_SAW_NAME] = (
        max(_dve_ops._SUB_OPCODE_FOR_NAME.values()) + 1
    )
else:
    SCALE_ADD_RANGE_WRAP = next(o for o in _dve_ops.OPS if o.name == _SAW_NAME)

_cache = {}


def _plan(charges, molIDs):
    """Host-side chunking plan from charges/molIDs (static per compile)."""
    charges = np.asarray(charges)
    molIDs = np.asarray(molIDs)
    assert np.all(np.diff(molIDs) >= 0), "molIDs must be sorted"
    perm = np.argsort(charges, kind="stable")
    mol_p = molIDs[perm]
    chg_p = charges[perm]

    # padded element groups
    counts = np.bincount(charges, minlength=NELEM)
    padded = [int(np.ceil(c / CH) * CH) for c in counts]
    A_pad = int(sum(padded))
    n_chunks = A_pad // CH

    # index into permuted arrays for each padded slot (-1 = padding)
    slot_idx = np.full(A_pad, -1, dtype=np.int64)
    src_off = 0
    dst_off = 0
    for e in range(NELEM):
        c = int(counts[e])
        slot_idx[dst_off:dst_off + c] = np.arange(src_off, src_off + c)
        src_off += c
        dst_off += padded[e]

    chunk_elem = []
    chunk_m0 = []
    W_need = 1
    for c in range(n_chunks):
        sl = slot_idx[c * CH:(c + 1) * CH]
        real = sl >= 0
        if real.any():
            mols = mol_p[sl[real]]
            t_lo = int(mols.min()) // 128
            t_hi = int(mols.max()) // 128
            W_need = max(W_need, t_hi - t_lo + 1)
            chunk_m0.append(t_lo)
            e = int(chg_p[sl[real][0]])
        else:
            chunk_m0.append(0)
            e = int(np.searchsorted(np.cumsum(padded), c * CH, side="right"))
        chunk_elem.append(e)
    W = W_need

    # nonzero (k-tile, wt) blocks of ST per chunk + ST data
    st_blocks = []   # list per chunk: list of (kt, wt) nonzero
    ST = np.zeros((n_chunks, CH, W * 128), dtype=np.float32)
    for c in range(n_chunks):
        sl = slot_idx[c * CH:(c + 1) * CH]
        real = np.nonzero(sl >= 0)[0]
        blocks = set()
        if len(real):
            ml = mol_p[sl[real]] - chunk_m0[c] * 128
            ok = (ml >= 0) & (ml < W * 128)
            ST[c, real[ok], ml[ok]] = 1.0
            for a, m in zip(real[ok], ml[ok]):
                blocks.add((int(a) // 128, int(m) // 128))
        st_blocks.append(sorted(blocks))

    chunk_real = [int((slot_idx[c * CH:(c + 1) * CH] >= 0).sum())
                  for c in range(n_chunks)]
    return dict(perm=perm, slot_idx=slot_idx, A_pad=A_pad, n_chunks=n_chunks,
                chunk_elem=chunk_elem, chunk_m0=chunk_m0, W=W, ST=ST,
                st_blocks=st_blocks, chunk_real=chunk_real)


def _build(plan):
    n_chunks = plan["n_chunks"]
    W = plan["W"]
    chunk_elem = plan["chunk_elem"]
    chunk_m0 = plan["chunk_m0"]
    st_blocks = plan["st_blocks"]

    nc = bacc.Bacc(num_devices=NCORES)
    gto_d = nc.dram_tensor("gto_swz", [n_chunks, 128, 4, CH], FP8, kind="ExternalInput")
    st_d = nc.dram_tensor("st_swz", [n_chunks, 128, 4 * W * 128], BF16, kind="ExternalInput")
    red_d = nc.dram_tensor("red_swz", [128, NELEM * 4, PROJ], FP8, kind="ExternalInput")
    w_d = nc.dram_tensor("w_swz", [128, NELEM * 2, NF_LOC], FP8, kind="ExternalInput")
    c_d = nc.dram_tensor("c_swz", [1, NELEM * NF_LOC], F32, kind="ExternalInput")
    y_d = nc.dram_tensor("y_swz", [128, MOLT], F32, kind="ExternalInput")
    ztz_d = nc.dram_tensor("ztz", [NF_LOC, NFEAT], BF16, kind="ExternalOutput")
    zty_d = nc.dram_tensor("zty", [NF_LOC, 1], F32, kind="ExternalOutput")

    with tile.TileContext(nc) as tc:
        with (
            tc.tile_pool(name="const", bufs=1) as constp,
            tc.tile_pool(name="zacc", bufs=1) as zaccp,
            tc.tile_pool(name="dram", bufs=1, space="DRAM") as dramp,
        ):
            red_sb = constp.tile([128, NELEM * 4, PROJ], FP8, tag="red")
            w_sb = constp.tile([128, NELEM * 2, NF_LOC], FP8, tag="w")
            c_sb = constp.tile([1, NELEM * NF_LOC], F32, tag="c")
            c_bc = constp.tile([128, NELEM * NF_LOC], F32, tag="cbc")
            y_sb = constp.tile([128, MOLT], F32, tag="y")
            # load order: red first (chunk-0 PT needs it), then the W slice for
            # the first element processed, then c/y, then the remaining W.
            order = sorted(range(n_chunks), key=lambda c: (chunk_m0[c], c))
            e_first = chunk_elem[order[0]]
            for q in [e_first] + [q for q in range(4) if q != e_first]:
                nc.sync.dma_start(out=red_sb[:, q * 4:(q + 1) * 4, :],
                                  in_=red_d[:, q * 4:(q + 1) * 4, :])
            nc.sync.dma_start(out=w_sb[:, e_first * 2:e_first * 2 + 2, :],
                              in_=w_d[:, e_first * 2:e_first * 2 + 2, :])
            nc.sync.dma_start(out=c_sb[:], in_=c_d[:])
            nc.gpsimd.partition_broadcast(c_bc[:], c_sb[:])
            nc.sync.dma_start(out=y_sb[:], in_=y_d[:])
            for e in range(NELEM):
                if e == e_first:
                    continue
                nc.sync.dma_start(out=w_sb[:, e * 2:e * 2 + 2, :],
                                  in_=w_d[:, e * 2:e * 2 + 2, :])

            z_sb = zaccp.tile([128, NMOL // 128 * NF_LOC], F32, tag="z")     # [128, 4096]
            zr_sb = zaccp.tile([128, MOLT, NF_LOC], FP8, tag="zr")
            nc.vector.memset(z_sb[:], 0.0)

            in_b = dramp.tile([NMOL, NF_LOC], FP8, tag="agin")
            GB = list(range(MOLT + 1))  # one mol tile per AG group
            NG = len(GB) - 1
            ag_bs = [
                dramp.tile([NCORES * (GB[g + 1] - GB[g]) * 128, NF_LOC], FP8,
                           addr_space="Shared", tag=f"agout{g}", name=f"ag_b_{g}")
                for g in range(NG)
            ]

            # ---------------- phase 1: chunks (m0-sorted) ----------------
            # last order-position touching each mol group
            group_last = [0] * NG
            for pos, ci in enumerate(order):
                if not st_blocks[ci]:
                    continue
                wts = {chunk_m0[ci] + wt for (kt, wt) in st_blocks[ci]}
                for mt in wts:
                    g = next(i for i in range(NG) if GB[i] <= mt < GB[i + 1])
                    group_last[g] = max(group_last[g], pos)
            for g in range(NG):  # groups complete monotonically
                group_last[g] = max(group_last[:g + 1])
            group_at = {}
            for g in range(NG):
                group_at.setdefault(group_last[g], []).append(g)

            _panel_cm = tc.tile_pool(name="panel", bufs=1)
            panelp = _panel_cm.__enter__()
            pan_all = panelp.tile([128, NCORES, MOLT, NF_LOC], FP8, tag="pan")

            def _mk_loader(g):
                def _ld():
                    nc.sync.dma_start(
                        out=pan_all[:, :, g, :],
                        in_=ag_bs[g][:].rearrange("(t p) c -> p t c", p=128),
                    )
                return _ld

            pang_loaders = {g: _mk_loader(g) for g in range(NG)}

            def emit_group_tail(g):
                for k in range(GB[g], GB[g + 1]):
                    nc.gpsimd.tensor_copy(
                        zr_sb[:, k, :],
                        z_sb[:, k * NF_LOC:(k + 1) * NF_LOC],
                    )
                    nc.sync.dma_start(
                        out=in_b[k * 128:(k + 1) * 128, :],
                        in_=zr_sb[:, k, :],
                    )
                nc.gpsimd.collective_compute(
                    "AllGather",
                    mybir.AluOpType.bypass,
                    replica_groups=[list(range(NCORES))],
                    ins=[in_b[GB[g] * 128:GB[g + 1] * 128, :].opt()],
                    outs=[ag_bs[g][:].opt()],
                )
                # prefetch the panel block of an AG that finished a while ago
                # (g-4 keeps the sync queue from blocking on the AG sem)
                if g - 4 in pang_loaders:
                    pang_loaders.pop(g - 4)()
                if g == NG - 1:
                    for gg in (4, 5):
                        if gg in pang_loaders:
                            pang_loaders.pop(gg)()

            with (
                tc.tile_pool(name="gtop", bufs=3) as gtop,
                tc.tile_pool(name="stp", bufs=3) as stp,
                tc.tile_pool(name="ptp", bufs=3) as ptp,
                tc.tile_pool(name="fp", bufs=3) as fpool,
                tc.tile_pool(name="ppt", bufs=2, space="PSUM") as ppt,
                tc.tile_pool(name="pf", bufs=3, space="PSUM") as pf,
                tc.tile_pool(name="pz", bufs=3, space="PSUM") as pz,
            ):
                n_real = plan["chunk_real"]
                contribs = {}
                for pos, ci in enumerate(order):
                    for (kt, wt) in sorted(set(st_blocks[ci])):
                        mt = chunk_m0[ci] + wt
                        lst = contribs.setdefault(mt, [])
                        if not lst or lst[-1] != pos:
                            lst.append(pos)
                zb_start = set()
                zb_stop = set()
                for mt, lst in contribs.items():
                    for j, pos in enumerate(lst):
                        if j % 2 == 0:
                            zb_start.add((mt, pos))
                        if j % 2 == 1 or j == len(lst) - 1:
                            zb_stop.add((mt, pos))
                z_ps_live = {}
                for pos, ci in enumerate(order):
                    e = chunk_elem[ci]
                    m0 = chunk_m0[ci]
                    if n_real[ci] == 0:
                        for g in group_at.get(pos, []):
                            emit_group_tail(g)
                        continue
                    mts = (n_real[ci] + 127) // 128   # live atom tiles
                    gto_t = gtop.tile([128, 4, CH], FP8, tag="gto")
                    nc.sync.dma_start(out=gto_t[:], in_=gto_d[ci, :, :, :])
                    st_t = stp.tile([128, 4 * W * 128], BF16, tag="st")
                    if st_blocks[ci]:
                        nc.sync.dma_start(out=st_t[:], in_=st_d[ci, :, :])

                    # PT [256, 512] (x16) -> pt_sb [128, 2, 512] fp8
                    pt_sb = ptp.tile([128, 2, CH], FP8, tag="pt")
                    for mp in range(2):  # proj tile
                        pt_ps = ppt.tile([128, CH], F32, tag="ptps")
                        for ktp in range(0, 4, 2):  # rep k tile pairs
                            nc.tensor.matmul(
                                pt_ps[:],
                                red_sb[:, e * 4 + ktp:e * 4 + ktp + 2,
                                       mp * 128:mp * 128 + 128],
                                gto_t[:, ktp:ktp + 2, :],
                                start=(ktp == 0), stop=(ktp == 2),
                                perf_mode=DR,
                            )
                        if pos % 2 == 0:
                            nc.scalar.copy(pt_sb[:, mp, :], pt_ps[:])
                        else:
                            nc.vector.tensor_copy(pt_sb[:, mp, :], pt_ps[:])

                    # feats F [512 atoms, 512 feats] -> f_sb [128, 4*512]
                    f_sb = fpool.tile([128, 4 * NF_LOC], BF16, tag="f")
                    for mt in range(mts):  # atom tile
                        f_ps = pf.tile([128, NF_LOC], F32, tag="fps")
                        nc.tensor.matmul(
                            f_ps[:],
                            pt_sb[:, 0:2, mt * 128:mt * 128 + 128],
                            w_sb[:, e * 2:e * 2 + 2, :],
                            start=True, stop=True,
                            perf_mode=DR,
                        )
                        fw = fpool.tile([128, NF_LOC], F32, tag="fw")
                        nc.vector._custom_dve(
                            SCALE_ADD_RANGE_WRAP, out=fw[:], in0=f_ps[:],
                            in1=c_bc[:, e * NF_LOC:(e + 1) * NF_LOC],
                            s0=float(1.0 / 256.0),
                            s1=float(np.pi), imm2=float(2 * np.pi),
                        )
                        nc.scalar.activation(
                            f_sb[:, mt * NF_LOC:(mt + 1) * NF_LOC], fw[:],
                            mybir.ActivationFunctionType.Sin,
                        )

                    # Z += ST^T @ F; PSUM tile accumulates 2 chunks per flush
                    for wt in range(W):
                        kts = [kt for (kt, w2) in st_blocks[ci] if w2 == wt]
                        if not kts:
                            continue
                        mt_out = m0 + wt
                        first = (mt_out, pos) in zb_start
                        last = (mt_out, pos) in zb_stop
                        if first:
                            z_ps_live[mt_out] = pz.tile([128, NF_LOC], F32, tag="zps", name=f"zps_{mt_out}_{pos}")
                        z_ps = z_ps_live[mt_out]
                        for i, kt in enumerate(kts):
                            nc.tensor.matmul(
                                z_ps[:],
                                st_t[:, (kt * W + wt) * 128:(kt * W + wt) * 128 + 128],
                                f_sb[:, kt * NF_LOC:(kt + 1) * NF_LOC],
                                start=(first and i == 0),
                                stop=(last and i == len(kts) - 1),
                            )
                        if last:
                            del z_ps_live[mt_out]
                            nc.vector.tensor_add(
                                z_sb[:, mt_out * NF_LOC:(mt_out + 1) * NF_LOC],
                                z_sb[:, mt_out * NF_LOC:(mt_out + 1) * NF_LOC],
                                z_ps[:],
                            )

                    for g in group_at.get(pos, []):
                        emit_group_tail(g)

            # ---------------- phase 2: ZTZ slice (A: k<6, B: k=6,7) ----------------
            with (
                tc.tile_pool(name="osb", bufs=NCORES) as osbp,
                tc.tile_pool(name="obsb", bufs=NCORES) as obsbp,
                tc.tile_pool(name="pztz", bufs=4, space="PSUM") as pztz,
                tc.tile_pool(name="pztzB", bufs=3, space="PSUM") as pztzB,
                tc.tile_pool(name="pzty", bufs=1, space="PSUM") as pzty,
            ):
                for g in sorted(pang_loaders):   # not prefetched during phase 1
                    pang_loaders.pop(g)()

                KA = 6  # k-tiles available once AG_0..AG_2 land (phase-1 end)
                o_sbs = {}
                for n in range(NCORES):
                    o_sb = osbp.tile([128, 4, NF_LOC], F32, tag="osb",
                                     name=f"o_sb_{n}")
                    o_sbs[n] = o_sb
                    for m in range(4):
                        ztz_ps = pztz.tile([128, NF_LOC], F32, tag="ztzpsA",
                                           name=f"ztzA_{n}_{m}")
                        for kp in range(0, KA, 2):
                            nc.tensor.matmul(
                                ztz_ps[:],
                                zr_sb[:, kp:kp + 2, m * 128:m * 128 + 128],
                                pan_all[:, n, kp:kp + 2, :],
                                start=(kp == 0), stop=(kp == KA - 2),
                                perf_mode=DR,
                            )
                        nc.scalar.copy(o_sb[:, m, :], ztz_ps[:])
                # ZtY on PE while the last gather flies
                zty_ps = pzty.tile([128, 4], F32, tag="ztyps")
                for m in range(4):
                    for k in range(MOLT):
                        nc.tensor.matmul(
                            zty_ps[:, m:m + 1],
                            z_sb[:, k * NF_LOC + m * 128: k * NF_LOC + m * 128 + 128],
                            y_sb[:, k:k + 1],
                            start=(k == 0), stop=(k == MOLT - 1),
                        )
                zty_sb = zaccp.tile([128, 4], F32, tag="ztysb")
                nc.vector.tensor_copy(zty_sb[:], zty_ps[:])
                nc.sync.dma_start(
                    out=zty_d[:].rearrange("(m p) o -> p (m o)", p=128),
                    in_=zty_sb[:],
                )
                for n in range(NCORES):
                    o_sb = o_sbs[n]
                    ob_sb = obsbp.tile([128, 4, NF_LOC], BF16, tag="obsb",
                                       name=f"ob_sb_{n}")
                    for m in range(4):
                        ztz_ps = pztzB.tile([128, NF_LOC], F32, tag="ztzpsB",
                                            name=f"ztzB_{n}_{m}")
                        nc.tensor.matmul(
                            ztz_ps[:],
                            zr_sb[:, KA:KA + 2, m * 128:m * 128 + 128],
                            pan_all[:, n, KA:KA + 2, :],
                            start=True, stop=True,
                            perf_mode=DR,
                        )
                        nc.vector.tensor_add(
                            ob_sb[:, m, :], o_sb[:, m, :], ztz_ps[:],
                        )
                    nc.sync.dma_start(
                        out=ztz_d[:, n * NF_LOC:(n + 1) * NF_LOC]
                            .rearrange("(m p) c -> p m c", p=128),
                        in_=ob_sb[:],
                    )
            _panel_cm.__exit__(None, None, None)
    nc.finalize()
    return nc


def _prep_inputs(gto, reductors, W_in, b, Y, plan):
    n_chunks = plan["n_chunks"]
    A_pad = plan["A_pad"]
    slot_idx = plan["slot_idx"]
    W = plan["W"]

    gto_p = np.zeros((A_pad, REP), dtype=np.float32)
    real = slot_idx >= 0
    gto_p[real] = np.asarray(gto)[plan["perm"][slot_idx[real]]]
    # [A_pad, REP] -> [n_chunks, 128(rep-part), 4, CH]
    gto_swz = np.ascontiguousarray(
        gto_p.reshape(n_chunks, CH, 4, 128).transpose(0, 3, 2, 1)
    ).astype(mybir.dt.np(mybir.dt.float8e4))

    st_swz = np.ascontiguousarray(
        plan["ST"].reshape(n_chunks, 4, 128, W * 128).transpose(0, 2, 1, 3)
    ).reshape(n_chunks, 128, 4 * W * 128).astype(mybir.dt.np(BF16))

    red_swz = np.ascontiguousarray(
        (np.asarray(reductors) * np.float32(16.0))
        .reshape(NELEM, 4, 128, PROJ).transpose(2, 0, 1, 3)
    ).reshape(128, NELEM * 4, PROJ).astype(mybir.dt.np(mybir.dt.float8e4))

    c_full = np.mod(np.asarray(b) + np.pi / 2 + np.pi, 2 * np.pi) - np.pi  # [-pi, pi)

    W_np = np.asarray(W_in)
    in_maps = []
    for d in range(NCORES):
        fsl = slice(d * NF_LOC, (d + 1) * NF_LOC)
        w_swz = np.ascontiguousarray(
            (W_np[:, :, fsl] * np.float32(16.0))
            .reshape(NELEM, 2, 128, NF_LOC).transpose(2, 0, 1, 3)
        ).reshape(128, NELEM * 2, NF_LOC)
        c_swz = np.ascontiguousarray(c_full[:, fsl]).reshape(1, NELEM * NF_LOC)
        in_maps.append({
            "gto_swz": gto_swz,
            "st_swz": st_swz,
            "red_swz": red_swz,
            "w_swz": w_swz.astype(mybir.dt.np(mybir.dt.float8e4)),
            "c_swz": c_swz.astype(np.float32),
            "y_swz": np.ascontiguousarray(
                np.asarray(Y).reshape(MOLT, 128).T
            ).astype(np.float32),
        })
    return in_maps


def _get_built(charges, molIDs):
    key = (hash(np.asarray(charges).tobytes()), hash(np.asarray(molIDs).tobytes()))
    if key not in _cache:
        plan = _plan(charges, molIDs)
        nc = _build(plan)
        _cache[key] = (plan, nc)
    return _cache[key]


def run(gto, reductors, W, b, Y, charges, molIDs, trace=False, tmpdir=None):
    plan, nc = _get_built(charges, molIDs)
    in_maps = _prep_inputs(gto, reductors, W, b, Y, plan)
    res = bass_utils.run_bass_kernel_spmd(
        nc, in_maps, core_ids=list(range(NCORES)), trace=trace, tmpdir=tmpdir,
    )
    scale2 = 2.0 / NFEAT
    scale = np.float32(np.sqrt(scale2))
    ztz = np.concatenate([res.results[d]["ztz"].astype(np.float32) for d in range(NCORES)], axis=0)
    zty = np.concatenate([res.results[d]["zty"] for d in range(NCORES)], axis=0)
    ztz = ztz * np.float32(scale2)
    ztz[np.arange(NFEAT), np.arange(NFEAT)] += np.float32(LLAMBDA)
    zty = zty * scale
    out = np.concatenate([ztz, zty], axis=1).astype(np.float32)
    return out, res


def kernel(gto, reductors, W, b, Y, charges, molIDs):
    out, _ = run(gto, reductors, W, b, Y, charges, molIDs)
    return out

